# revision 16
# baseline (speedup 1.0000x reference)
"""Trainium2 Bass kernel for the CNN-TRX few-shot attention head.

Sharding: data-parallel over the 200 queries (25 per NeuronCore); support set
and weights replicated per core.

  1. Frame projection in transposed layout f_T[d, frame], x/w in fp8-e4m3 via
     DoubleRow matmuls (2 k-chunks per pass); fp32 PSUM, bf16 f_b.
  2. Tuple gather (C(8,3)=56 frame triples) as 2-stage DVE column adds.
  3. LayerNorm of K projections column-wise: stats via ones-matmuls, DVE
     squares, two-pass emission so DVE squares don't serialize behind PE
     stat chains; the LN apply writes fp8 so score matmuls run fp8 DoubleRow.
  4. Support rows packed per class WITHOUT 128-padding (1400 rows, 11 chunks,
     split into two tiles so early score chunks start before the second LN
     window finishes). Per-class softmax sums use per-(class,chunk) 0/1 mask
     vectors as matmul stationaries; proto matmuls use class-masked exp
     copies on boundary chunks. All 25 queries gathered/LN'd in one batch
     (big DVE ops; per-group work reads column slices). d-chunks padded 9->10
     so score and A/B/C reduction matmuls run fully fp8-DoubleRow-paired.
  5. Per-class prototypes in T-layout; distance terms ||q_v||^2, <q_v,P>,
     ||P||^2, sum(exp) via ones-matmul column reductions; final combine on
     single-partition rows; logits = -sum_a dist / 56.
"""

import math
from itertools import combinations

import ml_dtypes
import numpy as np

SEQ = 8
IN_DIM = 2048
OUT_DIM = 1152
TSS = 3
WAY = 5
N_SUPPORT = 25
N_QUERIES = 200
PE_SCALE = 0.1
LN_EPS = 1e-5
T = 56
N_CORES = 8
NQL = N_QUERIES // N_CORES      # queries per core
G_Q = 5                         # queries per inner group
N_GROUPS = NQL // G_Q
C = G_Q * T                     # score columns per group (280)
NKCH = IN_DIM // 128            # 16
NDCH = OUT_DIM // 128           # 9
NMB = 6 * OUT_DIM // 128        # 54 projection column blocks
NX = SEQ * 2 * N_SUPPORT        # 400 frame columns per core
R = N_SUPPORT * T               # 1400 packed support rows
NWCH = (R + 127) // 128         # 11
RPAD = NWCH * 128               # 1408
WINS = ((0, 16), (16, 9))       # support item windows (16*56=896=7*128)
PAIRS = [(t0, t1) for t0 in range(SEQ - 2) for t1 in range(t0 + 1, SEQ - 1)]
LN_CHUNK = 448                  # LayerNorm column chunk (PSUM free-dim <= 512)
BF16 = ml_dtypes.bfloat16
FP8 = ml_dtypes.float8_e4m3

_CACHE = {}


def _pos_encoding():
    pos = np.arange(SEQ, dtype=np.float32)[:, None]
    div = np.exp(np.arange(0, IN_DIM, 2, dtype=np.float32) * -(math.log(10000.0) / IN_DIM))
    pe = np.zeros((SEQ, IN_DIM), dtype=np.float32)
    pe[:, 0::2] = np.sin(pos * div) * PE_SCALE
    pe[:, 1::2] = np.cos(pos * div) * PE_SCALE
    return pe


def _geom(counts):
    """Packed class layout: offs[c] row starts; spans[c] = [(w, lo, hi)]
    chunk windows each class covers (lo/hi partition rows within chunk)."""
    offs = [0]
    for c in range(WAY):
        offs.append(offs[-1] + int(counts[c]) * T)
    spans = []
    for c in range(WAY):
        w0, w1 = offs[c] // 128, (offs[c + 1] - 1) // 128
        spans.append([(w, max(0, offs[c] - 128 * w), min(128, offs[c + 1] - 128 * w))
                      for w in range(w0, w1 + 1)])
    return offs, spans


def _build_kernel(counts, trivial_gb, has_bias):
    import concourse.mybir as mybir
    import concourse.tile as tile
    from concourse import bacc
    from concourse.masks import make_identity

    f32 = mybir.dt.float32
    bf16 = mybir.dt.bfloat16
    fp8 = mybir.dt.float8e4
    AF = mybir.ActivationFunctionType
    ALU = mybir.AluOpType
    DR = mybir.MatmulPerfMode.DoubleRow
    offs, spans = _geom(counts)
    nmask = sum(len(s) for s in spans)
    inv_sqrt = 1.0 / math.sqrt(OUT_DIM)

    nc = bacc.Bacc("TRN2", target_bir_lowering=False, debug=False,
                   enable_asserts=False, num_devices=N_CORES)

    x_d = nc.dram_tensor("x", [128, NKCH, NX], fp8, kind="ExternalInput").ap()
    w_d = nc.dram_tensor("w", [128, NMB, NKCH, 128], fp8, kind="ExternalInput").ap()
    m_d = nc.dram_tensor("masks", [128, nmask], bf16, kind="ExternalInput").ap()
    if has_bias:
        pb_d = nc.dram_tensor("pb", [128, NMB], f32, kind="ExternalInput").ap()
    if not trivial_gb:
        g_d = nc.dram_tensor("lng", [128, NDCH], bf16, kind="ExternalInput").ap()
        b_d = nc.dram_tensor("lnb", [128, NDCH], bf16, kind="ExternalInput").ap()
    out_d = nc.dram_tensor("out", [NQL, WAY], f32, kind="ExternalOutput").ap()

    with tile.TileContext(nc) as tc:
        with tc.tile_pool(name="pers", bufs=1) as pers, \
             tc.tile_pool(name="small", bufs=1) as small, \
             tc.tile_pool(name="sprep", bufs=1) as sprep, \
             tc.tile_pool(name="pp_t", bufs=2, space="PSUM") as pp_t:
            # persistent tensors; support K kept as two tiles so score chunks
            # 0-6 only depend on the first LN window; d-chunk dim padded to 10
            # (chunk 9 zeroed) so fp8 DoubleRow pairs cover all chunks
            s_k8a = pers.tile([128, NDCH + 1, 7 * 128], fp8)
            s_k8b = pers.tile([128, NDCH + 1, RPAD - 7 * 128], fp8)
            s_v = pers.tile([128, NWCH, OUT_DIM], bf16)  # support V, row-natural
            qv_T = pers.tile([128, NDCH, NQL, T], bf16)  # all query V tuples
            qk8f = pers.tile([128, NDCH + 1, NQL * T], fp8)  # LN'd query K
            ones_sb = small.tile([128, 1], bf16)
            nc.vector.memset(ones_sb, 1.0)
            eps_sb = small.tile([1, 1], f32)
            nc.vector.memset(eps_sb, LN_EPS)
            mask_sb = small.tile([128, nmask], bf16)
            nc.sync.dma_start(mask_sb, m_d)
            mask_f = small.tile([128, nmask], f32)
            nc.vector.tensor_copy(mask_f, mask_sb)
            ident = small.tile([128, 128], bf16)
            make_identity(nc, ident)
            if has_bias:
                pb_sb = small.tile([128, NMB], f32)
                nc.sync.dma_start(pb_sb, pb_d)
            if not trivial_gb:
                g_sb = small.tile([128, NDCH], bf16)
                b_sb = small.tile([128, NDCH], bf16)
                nc.sync.dma_start(g_sb, g_d)
                nc.sync.dma_start(b_sb, b_d)
            logits5 = small.tile([WAY, NQL], f32)
            nc.vector.memset(s_k8a[:, NDCH], 0.0)
            nc.vector.memset(s_k8b[:, NDCH], 0.0)
            nc.vector.memset(qk8f[:, NDCH], 0.0)
            nc.vector.memset(s_k8b[:, :, R - 7 * 128:], 0.0)

            def sk8_pair(w, k2):
                ks = slice(2 * k2, 2 * k2 + 2)
                if w < 7:
                    return s_k8a[:, ks, w * 128:(w + 1) * 128]
                return s_k8b[:, ks, (w - 7) * 128:(w - 6) * 128]

            qkpool_cm = tc.tile_pool(name="qkpool", bufs=1)
            qkpool = qkpool_cm.__enter__()
            q_kT = qkpool.tile([128, NDCH, NQL, T], bf16)
            fb_cm = tc.tile_pool(name="fbpool", bufs=1)
            fbp = fb_cm.__enter__()
            f_b = [fbp.tile([128, NDCH, NX], fp8, name=f"f_b{j}") for j in range(6)]

            # ---------- Phase 1: frame projections (fp8 DoubleRow) ----------
            with tc.tile_pool(name="xt_pool", bufs=1) as xt_pool, \
                 tc.tile_pool(name="xw", bufs=2) as xw, \
                 tc.tile_pool(name="pp_proj", bufs=2, space="PSUM") as pp_proj:
                xt = xt_pool.tile([128, NKCH, NX], fp8)
                nc.sync.dma_start(xt, x_d)
                for m in range(NMB):
                    wm = xw.tile([128, NKCH, 128], fp8, tag="wslab")
                    nc.sync.dma_start(wm, w_d[:, m])
                    ps = pp_proj.tile([128, NX], f32, tag="projps")
                    for k2 in range(NKCH // 2):
                        nc.tensor.matmul(ps, wm[:, 2 * k2:2 * k2 + 2],
                                         xt[:, 2 * k2:2 * k2 + 2],
                                         start=(k2 == 0), stop=(k2 == NKCH // 2 - 1),
                                         perf_mode=DR)
                    dst = f_b[m // NDCH][:, m % NDCH]
                    if has_bias:
                        nc.scalar.activation(dst, ps, AF.Identity,
                                             bias=pb_sb[:, m:m + 1])
                    else:
                        nc.scalar.activation(dst, ps, AF.Copy)

            f_i = [fb.rearrange("p d (i s) -> p d i s", s=SEQ) for fb in f_b]

            def gather_one(dst4, kv, items0, n_items, pool, eng=None):
                """dst4 [128, NDCH, n_items, T] = tuple-gathered frame
                projections for one path (kv=0: K blocks 0-2, kv=1: V 3-5)."""
                eng = eng or nc.vector
                isl = slice(items0, items0 + n_items)
                b0, b1, b2 = (f_i[3 * kv + j] for j in range(TSS))
                p2 = pool.tile([128, NDCH, n_items, len(PAIRS)], bf16,
                               tag=f"pairs{kv}", name="p2", bufs=1)
                pi = 0
                for t0 in range(SEQ - 2):
                    run = SEQ - 2 - t0
                    a = b0[:, :, isl, t0:t0 + 1]
                    b = b1[:, :, isl, t0 + 1:t0 + 1 + run]
                    eng.tensor_add(p2[:, :, :, pi:pi + run],
                                   a.to_broadcast(b.shape), b)
                    pi += run
                ai = 0
                for pi, (t0, t1) in enumerate(PAIRS):
                    run = SEQ - 1 - t1
                    a = p2[:, :, :, pi:pi + 1]
                    b = b2[:, :, isl, t1 + 1:t1 + 1 + run]
                    eng.tensor_add(dst4[:, :, :, ai:ai + run],
                                   a.to_broadcast(b.shape), b)
                    ai += run

            def col_ln(raw, cols, out3, pool, psum_pool):
                """Column-wise LayerNorm of raw [128, NDCH, cols] (bf16,
                T-layout) -> out3 (fp8). Two-pass emission: all DVE squares +
                PE stat chains first, then the scalar/broadcast/apply chain."""
                state = []
                for c0 in range(0, cols, LN_CHUNK):
                    cw = min(LN_CHUNK, cols - c0)
                    r = raw[:, :, c0:c0 + cw]
                    sq = pool.tile([128, NDCH, cw], bf16, tag="lnsq", name="lnsq",
                                   bufs=2)
                    nc.vector.tensor_mul(sq, r, r)
                    ps_s = psum_pool.tile([1, cw], f32, tag="lnps", name="lnps")
                    ps_q = psum_pool.tile([1, cw], f32, tag="lnps", name="lnps")
                    for k in range(NDCH):
                        nc.tensor.matmul(ps_s, ones_sb, r[:, k],
                                         start=(k == 0), stop=(k == NDCH - 1))
                    for k in range(NDCH):
                        nc.tensor.matmul(ps_q, ones_sb, sq[:, k],
                                         start=(k == 0), stop=(k == NDCH - 1))
                    state.append((c0, cw, ps_s, ps_q))
                for c0, cw, ps_s, ps_q in state:
                    r = raw[:, :, c0:c0 + cw]
                    o = out3[:, :, c0:c0 + cw]
                    m_r = pool.tile([1, cw], f32, tag="lnm", name="lnm", bufs=1)
                    v_r = pool.tile([1, cw], f32, tag="lnv", name="lnv", bufs=1)
                    mm = pool.tile([1, cw], f32, tag="lnmm", name="lnmm", bufs=1)
                    nc.scalar.activation(m_r, ps_s, AF.Copy, scale=1.0 / OUT_DIM)
                    nc.scalar.activation(v_r, ps_q, AF.Copy, scale=1.0 / OUT_DIM)
                    nc.vector.tensor_mul(mm, m_r, m_r)
                    nc.vector.tensor_sub(v_r, v_r, mm)
                    nc.scalar.activation(v_r, v_r, AF.Sqrt, bias=eps_sb)
                    nc.vector.reciprocal_approx_accurate(v_r, v_r, mm)
                    m_h = pool.tile([1, cw], bf16, tag="lnmh", name="lnmh", bufs=1)
                    v_h = pool.tile([1, cw], bf16, tag="lnvh", name="lnvh", bufs=1)
                    nc.vector.tensor_copy(m_h, m_r)
                    nc.vector.tensor_copy(v_h, v_r)
                    m_b = pool.tile([128, cw], bf16, tag="lnmb", name="lnmb", bufs=1)
                    a_b = pool.tile([128, cw], bf16, tag="lnab", name="lnab", bufs=1)
                    nc.gpsimd.partition_broadcast(m_b, m_h)
                    nc.gpsimd.partition_broadcast(a_b, v_h)
                    mb3 = m_b[:, None, :].to_broadcast([128, NDCH, cw])
                    ab3 = a_b[:, None, :].to_broadcast([128, NDCH, cw])
                    nc.vector.tensor_sub(r, r, mb3)
                    if trivial_gb:
                        nc.vector.tensor_mul(o, r, ab3)
                    else:
                        nc.vector.tensor_mul(r, r, ab3)
                        for k in range(NDCH):
                            nc.vector.tensor_scalar(o[:, k], r[:, k],
                                                    g_sb[:, k:k + 1], b_sb[:, k:k + 1],
                                                    ALU.mult, ALU.add)

            # ---------- Phase 2a: support K -> LN -> fp8 (two windows) -------
            with tc.tile_pool(name="pp_ln", bufs=2, space="PSUM") as pp_ln:
                kws = []
                for wi, (it0, n_it) in enumerate(WINS):
                    cols = n_it * T
                    nch = (cols + 127) // 128
                    kw_t = sprep.tile([128, NDCH, nch * 128], bf16,
                                      tag=f"win{wi}", name="kwin", bufs=1)
                    gather_one(kw_t[:, :, :cols].rearrange("p m (n a) -> p m n a",
                                                           a=T),
                               0, it0, n_it, sprep)
                    kws.append((kw_t, cols))
                gather_one(q_kT, 0, N_SUPPORT, NQL, sprep)
                gather_one(qv_T, 1, N_SUPPORT, NQL, sprep, eng=nc.gpsimd)
                for wi, (kw_t, cols) in enumerate(kws):
                    dst = s_k8a if wi == 0 else s_k8b
                    col_ln(kw_t[:, :, :cols], cols, dst[:, :NDCH, :cols], sprep,
                           pp_ln)
                qkf = q_kT.rearrange("p m n a -> p m (n a)")
                col_ln(qkf, NQL * T, qk8f[:, :NDCH], sprep, pp_ln)
            fb_last = f_b  # keep name alive; fbpool closed below

            # ---------- Phase 2c: support V gather + transpose ----------
            # gathers emitted now (f_b still open); transposes emitted inside
            # group 0 so group-0 scores/sums don't queue behind them on the PE
            vwins = []
            for wi, (it0, n_it) in enumerate(WINS):
                cols = n_it * T
                nch = (cols + 127) // 128
                wlo = (it0 * T) // 128
                vw_t = sprep.tile([128, NDCH, nch * 128], bf16,
                                  tag=f"win{wi}", name="vwin", bufs=1)
                if cols < nch * 128:
                    nc.vector.memset(vw_t[:, :, cols:], 0.0)
                gather_one(vw_t[:, :, :cols].rearrange("p m (n a) -> p m n a",
                                                       a=T),
                           1, it0, n_it, sprep, eng=nc.gpsimd)
                vwins.append((vw_t, nch, wlo))
            fb_cm.__exit__(None, None, None)
            qkpool_cm.__exit__(None, None, None)

            def emit_support_v():
                for vw_t, nch, wlo in vwins:
                    for w in range(nch):
                        for dd in range(NDCH):
                            ps = pp_t.tile([128, 128], bf16, tag="tps")
                            nc.tensor.transpose(
                                ps, vw_t[:, dd, w * 128:(w + 1) * 128], ident)
                            if dd % 2 == 0:
                                nc.vector.tensor_copy(
                                    s_v[:, wlo + w, dd * 128:(dd + 1) * 128], ps)
                            else:
                                nc.scalar.activation(
                                    s_v[:, wlo + w, dd * 128:(dd + 1) * 128],
                                    ps, AF.Copy)

            # mask column index per (class, chunk)
            mcol = {}
            mi = 0
            for c in range(WAY):
                for (w, lo, hi) in spans[c]:
                    mcol[(c, w)] = mi
                    mi += 1

            # ---------- Phase 3: per-group query pipeline ----------
            TS, TB, TC, TA = 0, 1, 2, 3  # term slots in the packed tile

            def packed_sum(ps_tile, slot, rhs_chunks, first, last):
                """Accumulate masked column sums into ps_tile[32*slot] using a
                col-group tile_position; rhs_chunks = [(stat_col, mov)]."""
                out = ps_tile[32 * slot:32 * slot + 1]
                for i, (stat, rhs) in enumerate(rhs_chunks):
                    nc.tensor.matmul(out, stat, rhs, start=(first and i == 0),
                                     stop=(last and i == len(rhs_chunks) - 1),
                                     tile_position=(0, 32 * slot),
                                     skip_group_check=True)

            def chain10(ps_tile, slot, src):
                """Column sum of src [128, NDCH, C] via a 9-link ones chain
                into ps_tile[32*slot]."""
                packed_sum(ps_tile, slot, [(ones_sb, src[:, k])
                                           for k in range(NDCH)], True, True)

            with tc.tile_pool(name="grp", bufs=2) as grp, \
                 tc.tile_pool(name="rows", bufs=2) as rows_pool, \
                 tc.tile_pool(name="pp_sc", bufs=2, space="PSUM") as pp_sc, \
                 tc.tile_pool(name="pp_pr", bufs=2, space="PSUM") as pp_pr, \
                 tc.tile_pool(name="pp_row", bufs=2, space="PSUM") as pp_row:
                for g in range(N_GROUPS):
                    gsl = slice(g * C, (g + 1) * C)
                    qv3 = qv_T[:, :, g * G_Q:(g + 1) * G_Q, :].rearrange(
                        "p m q a -> p m (q a)")

                    # scoresT + exp (fp8 DoubleRow over 10 padded d-chunks)
                    exp_t = grp.tile([128, NWCH, C], bf16, tag="exp", bufs=2)
                    for w in range(NWCH):
                        ps = pp_sc.tile([128, C], f32, tag="scps")
                        for k2 in range(5):
                            nc.tensor.matmul(ps, sk8_pair(w, k2),
                                             qk8f[:, 2 * k2:2 * k2 + 2, gsl],
                                             start=(k2 == 0), stop=(k2 == 4),
                                             perf_mode=DR)
                        nc.scalar.activation(exp_t[:, w], ps, AF.Exp, scale=inv_sqrt)

                    packed = rows_pool.tile([WAY, 4, C], f32, tag="packed")

                    # A = ||q_v||^2 per column
                    qsq = grp.tile([128, NDCH, C], bf16, tag="qsq", bufs=1)
                    nc.vector.tensor_mul(qsq, qv3, qv3)
                    ps_a = pp_row.tile([128, C], f32, tag="sumps", name="ps_a")
                    chain10(ps_a, 0, qsq)
                    a_sb = rows_pool.tile([1, C], f32, tag="a_sb", bufs=1)
                    nc.vector.tensor_copy(a_sb, ps_a[0:1])
                    a5 = rows_pool.tile([WAY, C], f32, tag="a5", bufs=1)
                    nc.gpsimd.partition_broadcast(a5, a_sb)

                    def s_chunks(c):
                        return [(mask_sb[:, mcol[(c, w)]:mcol[(c, w)] + 1],
                                 exp_t[:, w]) for (w, lo, hi) in spans[c]]

                    def stage_rows(ps, rows):
                        """PSUM sum-tile -> SBUF (one lane-parallel copy), then
                        DMA rows {32j} into packed[class, term]."""
                        st = rows_pool.tile([128, C], f32, tag="stage", name="stage")
                        nc.scalar.activation(st, ps, AF.Copy)
                        st4 = st.rearrange("(j z) n -> j z n", z=32)
                        for j, (cc, term) in enumerate(rows):
                            nc.sync.dma_start(packed[cc:cc + 1, term],
                                              st4[j:j + 1, 0, :])

                    # S_0..S_3 packed in one PSUM tile (concurrent col-groups)
                    ps_s03 = pp_row.tile([128, C], f32, tag="sumps", name="ps_s03")
                    nchunks = max(len(spans[c]) for c in range(4))
                    for i in range(nchunks):
                        for c in range(4):
                            ch = s_chunks(c)
                            if i < len(ch):
                                packed_sum(ps_s03, c, [ch[i]], i == 0,
                                           i == len(ch) - 1)
                    stage_rows(ps_s03, [(0, TS), (1, TS), (2, TS), (3, TS)])

                    # remaining sum streams: S_4, then B_c/C_c per class,
                    # packed 4 per PSUM tile
                    ps_bc = pp_row.tile([128, C], f32, tag="sumps", name="ps_bc0")
                    packed_sum(ps_bc, 0, s_chunks(4), True, True)
                    pending = [(4, TS)]

                    if g == 0:
                        emit_support_v()

                    def bc_flush(force=False):
                        nonlocal ps_bc, pending
                        if pending and (force or len(pending) >= 3):
                            stage_rows(ps_bc, pending)
                            pending = []
                            if not force:
                                ps_bc = pp_row.tile([128, C], f32, tag="sumps",
                                                    name="ps_bc")
                    for c in range(WAY):
                        # prototypes: P[d, col] accumulated over class rows;
                        # boundary chunks use class-masked exp copies
                        movs = []
                        for (w, lo, hi) in spans[c]:
                            if lo == 0 and hi == 128:
                                movs.append((s_v[:, w], exp_t[:, w]))
                            else:
                                em = grp.tile([128, C], bf16, tag="emask",
                                              name="emask")
                                nc.vector.tensor_scalar(
                                    em, exp_t[:, w],
                                    mask_f[:, mcol[(c, w)]:mcol[(c, w)] + 1],
                                    0.0, ALU.mult, ALU.add)
                                movs.append((s_v[:, w], em))
                        pcl = grp.tile([128, NDCH, C], bf16, tag="pcl", bufs=1)
                        pt = grp.tile([128, NDCH, C], bf16, tag="pt", bufs=1)
                        p2t = grp.tile([128, NDCH, C], bf16, tag="p2t", bufs=1)
                        for dd in range(NDCH):
                            ps_p = pp_pr.tile([128, C], f32, tag="prps")
                            for wi, (sv_w, mov) in enumerate(movs):
                                nc.tensor.matmul(ps_p,
                                                 sv_w[:, dd * 128:(dd + 1) * 128],
                                                 mov, start=(wi == 0),
                                                 stop=(wi == len(movs) - 1))
                            if dd % 2 == 0:
                                nc.vector.tensor_copy(pcl[:, dd], ps_p)
                            else:
                                nc.scalar.activation(pcl[:, dd], ps_p, AF.Copy)
                        nc.vector.tensor_mul(pt, pcl, qv3)
                        nc.vector.tensor_mul(p2t, pcl, pcl)
                        sb, sc = len(pending), len(pending) + 1
                        chain10(ps_bc, sb, pt)
                        chain10(ps_bc, sc, p2t)
                        pending += [(c, TB), (c, TC)]
                        bc_flush()
                    bc_flush(force=True)

                    # dist = A - 2 B/S + C/S^2 ; logits = -sum_a dist / T
                    sinv = rows_pool.tile([WAY, C], f32, tag="sinv", bufs=1)
                    sscr = rows_pool.tile([WAY, C], f32, tag="sscr", bufs=1)
                    nc.vector.reciprocal_approx_accurate(sinv, packed[:, TS], sscr)
                    u = rows_pool.tile([WAY, C], f32, tag="u", bufs=1)
                    nc.vector.tensor_mul(u, packed[:, TC, :], sinv)
                    nc.vector.scalar_tensor_tensor(u, packed[:, TB, :], -2.0, u,
                                                   ALU.mult, ALU.add)
                    nc.vector.tensor_mul(u, u, sinv)
                    nc.vector.tensor_add(u, u, a5)
                    u4 = u.rearrange("w (q a) -> w q a", a=T)
                    red = rows_pool.tile([WAY, G_Q], f32, tag="red", bufs=1)
                    nc.vector.reduce_sum(red, u4, mybir.AxisListType.X)
                    nc.scalar.activation(logits5[:, g * G_Q:(g + 1) * G_Q], red,
                                         AF.Copy, scale=-1.0 / T)

            nc.sync.dma_start(out_d.rearrange("q c -> c q"), logits5)

    nc.compile()
    return nc


def kernel(support_set, support_labels, queries, k_w, k_b, v_w, v_b, ln_g, ln_b):
    import concourse.bass_utils as bass_utils

    support_set = np.asarray(support_set, dtype=np.float32)
    queries = np.asarray(queries, dtype=np.float32)
    labels = np.asarray(support_labels, dtype=np.int32)
    k_w = np.asarray(k_w, dtype=np.float32)
    v_w = np.asarray(v_w, dtype=np.float32)
    k_b = np.asarray(k_b, dtype=np.float32)
    v_b = np.asarray(v_b, dtype=np.float32)
    ln_g = np.asarray(ln_g, dtype=np.float32)
    ln_b = np.asarray(ln_b, dtype=np.float32)

    pe = _pos_encoding()
    s = support_set + pe[None]
    q = queries + pe[None]
    order = np.argsort(labels, kind="stable")
    counts = np.bincount(labels, minlength=WAY)
    s_sorted = s[order]
    trivial_gb = bool(np.all(ln_g == 1.0) and np.all(ln_b == 0.0))
    has_bias = bool(np.any(k_b != 0.0) or np.any(v_b != 0.0))

    key = (tuple(int(x) for x in counts), trivial_gb, has_bias)
    if key not in _CACHE:
        _CACHE[key] = _build_kernel(counts, trivial_gb, has_bias)
    nc = _CACHE[key]

    W = np.zeros((IN_DIM, 6 * OUT_DIM), np.float32)
    for j in range(TSS):
        W[:, j * OUT_DIM:(j + 1) * OUT_DIM] = k_w[j * IN_DIM:(j + 1) * IN_DIM]
        W[:, (TSS + j) * OUT_DIM:(TSS + j + 1) * OUT_DIM] = v_w[j * IN_DIM:(j + 1) * IN_DIM]
    w_perm = np.ascontiguousarray(
        W.reshape(NKCH, 128, NMB, 128).transpose(1, 2, 0, 3)).astype(FP8)

    offs, spans = _geom(counts)
    nmask = sum(len(sp) for sp in spans)
    masks = np.zeros((128, nmask), np.float32)
    mi = 0
    for c in range(WAY):
        for (w, lo, hi) in spans[c]:
            masks[lo:hi, mi] = 1.0
            mi += 1
    masks = masks.astype(BF16)

    base = {"w": w_perm, "masks": masks}
    if has_bias:
        pb = np.zeros((128, NMB), np.float32)
        for m in range(NMB):
            blk, dd = divmod(m, NDCH)
            bias = k_b if blk < TSS else v_b
            pb[:, m] = bias[dd * 128:(dd + 1) * 128] / TSS
        base["pb"] = pb
    if not trivial_gb:
        base["lng"] = np.ascontiguousarray(ln_g.reshape(NDCH, 128).T).astype(BF16)
        base["lnb"] = np.ascontiguousarray(ln_b.reshape(NDCH, 128).T).astype(BF16)

    in_maps = []
    for core in range(N_CORES):
        qs = q[core * NQL:(core + 1) * NQL]
        X = np.concatenate([s_sorted.reshape(-1, IN_DIM), qs.reshape(-1, IN_DIM)], 0)
        x_perm = np.ascontiguousarray(
            X.T.reshape(NKCH, 128, NX).transpose(1, 0, 2)).astype(FP8)
        in_maps.append(dict(base, x=x_perm))

    global _LAST_IN_MAPS
    _LAST_IN_MAPS = in_maps
    res = bass_utils.run_bass_kernel_spmd(nc, in_maps, core_ids=list(range(N_CORES)))
    return np.concatenate([res.results[i]["out"] for i in range(N_CORES)], 0)


_LAST_IN_MAPS = None


# revision 17
# speedup vs baseline: 1.2264x; 1.2264x over previous
"""Trainium2 Bass kernel for the CNN-TRX few-shot attention head.

Sharding: data-parallel over the 200 queries (25 per NeuronCore); support set
and weights replicated per core.

  1. Frame projection in transposed layout f_T[d, frame], x/w in fp8-e4m3 via
     DoubleRow matmuls (2 k-chunks per pass); fp32 PSUM, bf16 f_b.
  2. Tuple gather (C(8,3)=56 frame triples) as 2-stage DVE column adds.
  3. LayerNorm of K projections column-wise: stats via ones-matmuls, DVE
     squares, two-pass emission so DVE squares don't serialize behind PE
     stat chains; the LN apply writes fp8 so score matmuls run fp8 DoubleRow.
  4. Support rows packed per class WITHOUT 128-padding (1400 rows, 11 chunks,
     split into two tiles so early score chunks start before the second LN
     window finishes). Per-class softmax sums use per-(class,chunk) 0/1 mask
     vectors as matmul stationaries; proto matmuls use class-masked exp
     copies on boundary chunks.
  5. Per-class prototypes in T-layout; distance terms ||q_v||^2, <q_v,P>,
     ||P||^2, sum(exp) via ones-matmul column reductions; final combine on
     single-partition rows; logits = -sum_a dist / 56.
"""

import math
from itertools import combinations

import ml_dtypes
import numpy as np

SEQ = 8
IN_DIM = 2048
OUT_DIM = 1152
TSS = 3
WAY = 5
N_SUPPORT = 25
N_QUERIES = 200
PE_SCALE = 0.1
LN_EPS = 1e-5
T = 56
N_CORES = 8
NQL = N_QUERIES // N_CORES      # queries per core
G_Q = 5                         # queries per inner group
N_GROUPS = NQL // G_Q
C = G_Q * T                     # score columns per group (280)
NKCH = IN_DIM // 128            # 16
NDCH = OUT_DIM // 128           # 9
NMB = 6 * OUT_DIM // 128        # 54 projection column blocks
NX = SEQ * 2 * N_SUPPORT        # 400 frame columns per core
R = N_SUPPORT * T               # 1400 packed support rows
NWCH = (R + 127) // 128         # 11
RPAD = NWCH * 128               # 1408
WINS = ((0, 16), (16, 9))       # support item windows (16*56=896=7*128)
PAIRS = [(t0, t1) for t0 in range(SEQ - 2) for t1 in range(t0 + 1, SEQ - 1)]
LN_CHUNK = 448                  # LayerNorm column chunk (PSUM free-dim <= 512)
BF16 = ml_dtypes.bfloat16
FP8 = ml_dtypes.float8_e4m3

_CACHE = {}


def _pos_encoding():
    pos = np.arange(SEQ, dtype=np.float32)[:, None]
    div = np.exp(np.arange(0, IN_DIM, 2, dtype=np.float32) * -(math.log(10000.0) / IN_DIM))
    pe = np.zeros((SEQ, IN_DIM), dtype=np.float32)
    pe[:, 0::2] = np.sin(pos * div) * PE_SCALE
    pe[:, 1::2] = np.cos(pos * div) * PE_SCALE
    return pe


def _geom(counts):
    """Packed class layout: offs[c] row starts; spans[c] = [(w, lo, hi)]
    chunk windows each class covers (lo/hi partition rows within chunk)."""
    offs = [0]
    for c in range(WAY):
        offs.append(offs[-1] + int(counts[c]) * T)
    spans = []
    for c in range(WAY):
        w0, w1 = offs[c] // 128, (offs[c + 1] - 1) // 128
        spans.append([(w, max(0, offs[c] - 128 * w), min(128, offs[c + 1] - 128 * w))
                      for w in range(w0, w1 + 1)])
    return offs, spans


def _build_kernel(counts, trivial_gb, has_bias):
    import concourse.mybir as mybir
    import concourse.tile as tile
    from concourse import bacc
    from concourse.masks import make_identity

    f32 = mybir.dt.float32
    bf16 = mybir.dt.bfloat16
    fp8 = mybir.dt.float8e4
    AF = mybir.ActivationFunctionType
    ALU = mybir.AluOpType
    DR = mybir.MatmulPerfMode.DoubleRow
    offs, spans = _geom(counts)
    nmask = sum(len(s) for s in spans)
    inv_sqrt = 1.0 / math.sqrt(OUT_DIM)

    nc = bacc.Bacc("TRN2", target_bir_lowering=False, debug=False,
                   enable_asserts=False, num_devices=N_CORES)

    x_d = nc.dram_tensor("x", [128, NKCH, NX], fp8, kind="ExternalInput").ap()
    w_d = nc.dram_tensor("w", [128, NMB, NKCH, 128], fp8, kind="ExternalInput").ap()
    m_d = nc.dram_tensor("masks", [128, nmask], bf16, kind="ExternalInput").ap()
    if has_bias:
        pb_d = nc.dram_tensor("pb", [128, NMB], f32, kind="ExternalInput").ap()
    if not trivial_gb:
        g_d = nc.dram_tensor("lng", [128, NDCH], bf16, kind="ExternalInput").ap()
        b_d = nc.dram_tensor("lnb", [128, NDCH], bf16, kind="ExternalInput").ap()
    out_d = nc.dram_tensor("out", [NQL, WAY], f32, kind="ExternalOutput").ap()

    with tile.TileContext(nc) as tc:
        with tc.tile_pool(name="pers", bufs=1) as pers, \
             tc.tile_pool(name="small", bufs=1) as small, \
             tc.tile_pool(name="sprep", bufs=1) as sprep, \
             tc.tile_pool(name="pp_t", bufs=2, space="PSUM") as pp_t:
            # persistent tensors; support K kept as two tiles so score chunks
            # 0-6 only depend on the first LN window
            f_b = [pers.tile([128, NDCH, NX], fp8, name=f"f_b{j}") for j in range(6)]
            s_k8a = pers.tile([128, NDCH, 7 * 128], fp8)
            s_k8b = pers.tile([128, NDCH, RPAD - 7 * 128], fp8)
            s_v = pers.tile([128, NWCH, OUT_DIM], bf16)  # support V, row-natural
            ones_sb = small.tile([128, 1], bf16)
            nc.vector.memset(ones_sb, 1.0)
            eps_sb = small.tile([1, 1], f32)
            nc.vector.memset(eps_sb, LN_EPS)
            mask_sb = small.tile([128, nmask], bf16)
            nc.sync.dma_start(mask_sb, m_d)
            mask_f = small.tile([128, nmask], f32)
            nc.vector.tensor_copy(mask_f, mask_sb)
            ident = small.tile([128, 128], bf16)
            make_identity(nc, ident)
            if has_bias:
                pb_sb = small.tile([128, NMB], f32)
                nc.sync.dma_start(pb_sb, pb_d)
            if not trivial_gb:
                g_sb = small.tile([128, NDCH], bf16)
                b_sb = small.tile([128, NDCH], bf16)
                nc.sync.dma_start(g_sb, g_d)
                nc.sync.dma_start(b_sb, b_d)
            logits5 = small.tile([WAY, NQL], f32)

            def sk8_chunk(w, ks):
                if w < 7:
                    return s_k8a[:, ks, w * 128:(w + 1) * 128]
                return s_k8b[:, ks, (w - 7) * 128:(w - 6) * 128]

            # ---------- Phase 1: frame projections (fp8 DoubleRow) ----------
            with tc.tile_pool(name="xt_pool", bufs=1) as xt_pool, \
                 tc.tile_pool(name="xw", bufs=3) as xw, \
                 tc.tile_pool(name="pp_proj", bufs=2, space="PSUM") as pp_proj:
                xt = xt_pool.tile([128, NKCH, NX], fp8)
                nc.sync.dma_start(xt, x_d)
                for m in range(NMB):
                    wm = xw.tile([128, NKCH, 128], fp8, tag="wslab")
                    nc.sync.dma_start(wm, w_d[:, m])
                    ps = pp_proj.tile([128, NX], f32, tag="projps")
                    for k2 in range(NKCH // 2):
                        nc.tensor.matmul(ps, wm[:, 2 * k2:2 * k2 + 2],
                                         xt[:, 2 * k2:2 * k2 + 2],
                                         start=(k2 == 0), stop=(k2 == NKCH // 2 - 1),
                                         perf_mode=DR)
                    dst = f_b[m // NDCH][:, m % NDCH]
                    if has_bias:
                        nc.scalar.activation(dst, ps, AF.Identity,
                                             bias=pb_sb[:, m:m + 1])
                    else:
                        nc.scalar.activation(dst, ps, AF.Copy)

            f_i = [fb.rearrange("p d (i s) -> p d i s", s=SEQ) for fb in f_b]

            def gather_one(dst4, kv, items0, n_items, pool):
                """dst4 [128, NDCH, n_items, T] = tuple-gathered frame
                projections for one path (kv=0: K blocks 0-2, kv=1: V 3-5)."""
                isl = slice(items0, items0 + n_items)
                b0, b1, b2 = (f_i[3 * kv + j] for j in range(TSS))
                p2 = pool.tile([128, NDCH, n_items, len(PAIRS)], bf16,
                               tag=f"pairs{kv}", name="p2", bufs=1)
                pi = 0
                for t0 in range(SEQ - 2):
                    run = SEQ - 2 - t0
                    a = b0[:, :, isl, t0:t0 + 1]
                    b = b1[:, :, isl, t0 + 1:t0 + 1 + run]
                    nc.vector.tensor_add(p2[:, :, :, pi:pi + run],
                                         a.to_broadcast(b.shape), b)
                    pi += run
                ai = 0
                for pi, (t0, t1) in enumerate(PAIRS):
                    run = SEQ - 1 - t1
                    a = p2[:, :, :, pi:pi + 1]
                    b = b2[:, :, isl, t1 + 1:t1 + 1 + run]
                    nc.vector.tensor_add(dst4[:, :, :, ai:ai + run],
                                         a.to_broadcast(b.shape), b)
                    ai += run

            def col_ln(raw, cols, out3, pool, psum_pool, ptag="lnps"):
                """Column-wise LayerNorm of raw [128, NDCH, cols] (bf16,
                T-layout) -> out3 (fp8). Two-pass emission: all DVE squares +
                PE stat chains first, then the scalar/broadcast/apply chain,
                so squares of later chunks don't queue behind applies."""
                state = []
                for c0 in range(0, cols, LN_CHUNK):
                    cw = min(LN_CHUNK, cols - c0)
                    r = raw[:, :, c0:c0 + cw]
                    sq = pool.tile([128, NDCH, cw], bf16, tag="lnsq", name="lnsq",
                                   bufs=2)
                    nc.vector.tensor_mul(sq, r, r)
                    ps_s = psum_pool.tile([1, cw], f32, tag=ptag, name="lnps")
                    ps_q = psum_pool.tile([1, cw], f32, tag=ptag, name="lnps")
                    for k in range(NDCH):
                        nc.tensor.matmul(ps_s, ones_sb, r[:, k],
                                         start=(k == 0), stop=(k == NDCH - 1))
                    for k in range(NDCH):
                        nc.tensor.matmul(ps_q, ones_sb, sq[:, k],
                                         start=(k == 0), stop=(k == NDCH - 1))
                    state.append((c0, cw, ps_s, ps_q))
                for c0, cw, ps_s, ps_q in state:
                    r = raw[:, :, c0:c0 + cw]
                    o = out3[:, :, c0:c0 + cw]
                    m_r = pool.tile([1, cw], f32, tag="lnm", name="lnm", bufs=2)
                    v_r = pool.tile([1, cw], f32, tag="lnv", name="lnv", bufs=2)
                    mm = pool.tile([1, cw], f32, tag="lnmm", name="lnmm", bufs=2)
                    nc.scalar.activation(m_r, ps_s, AF.Copy, scale=1.0 / OUT_DIM)
                    nc.scalar.activation(v_r, ps_q, AF.Copy, scale=1.0 / OUT_DIM)
                    nc.vector.tensor_mul(mm, m_r, m_r)
                    nc.vector.tensor_sub(v_r, v_r, mm)
                    nc.scalar.activation(v_r, v_r, AF.Sqrt, bias=eps_sb)
                    nc.vector.reciprocal_approx_accurate(v_r, v_r, mm)
                    m_h = pool.tile([1, cw], bf16, tag="lnmh", name="lnmh", bufs=2)
                    v_h = pool.tile([1, cw], bf16, tag="lnvh", name="lnvh", bufs=2)
                    nc.vector.tensor_copy(m_h, m_r)
                    nc.vector.tensor_copy(v_h, v_r)
                    m_b = pool.tile([128, cw], bf16, tag="lnmb", name="lnmb", bufs=2)
                    a_b = pool.tile([128, cw], bf16, tag="lnab", name="lnab", bufs=2)
                    nc.gpsimd.partition_broadcast(m_b, m_h)
                    nc.gpsimd.partition_broadcast(a_b, v_h)
                    mb3 = m_b[:, None, :].to_broadcast([128, NDCH, cw])
                    ab3 = a_b[:, None, :].to_broadcast([128, NDCH, cw])
                    nc.vector.tensor_sub(r, r, mb3)
                    if trivial_gb:
                        nc.vector.tensor_mul(o, r, ab3)
                    else:
                        nc.vector.tensor_mul(r, r, ab3)
                        for k in range(NDCH):
                            nc.vector.tensor_scalar(o[:, k], r[:, k],
                                                    g_sb[:, k:k + 1], b_sb[:, k:k + 1],
                                                    ALU.mult, ALU.add)

            # ---------- Phase 2a: support K -> LN -> fp8 (two windows) -------
            nc.vector.memset(s_k8b[:, :, R - 7 * 128:], 0.0)
            with tc.tile_pool(name="pp_ln", bufs=2, space="PSUM") as pp_ln:
                for wi, (it0, n_it) in enumerate(WINS):
                    cols = n_it * T
                    kw_t = sprep.tile([128, NDCH, 7 * 128], bf16, tag="swin",
                                      name="kwin", bufs=1)
                    gather_one(kw_t[:, :, :cols].rearrange("p m (n a) -> p m n a",
                                                           a=T),
                               0, it0, n_it, sprep)
                    dst = s_k8a if wi == 0 else s_k8b
                    col_ln(kw_t[:, :, :cols], cols, dst[:, :, :cols], sprep, pp_ln)

            # ---------- Phase 2b: group-0 queries (overlap support V) -------
            grp_cm = tc.tile_pool(name="grp", bufs=2)
            rows_cm = tc.tile_pool(name="rows", bufs=2)
            grp = grp_cm.__enter__()
            rows_pool = rows_cm.__enter__()

            def q_gather(g, psum_pool):
                q_kT = grp.tile([128, NDCH, G_Q, T], bf16, tag="qk", bufs=1)
                q_vT = grp.tile([128, NDCH, G_Q, T], bf16, tag="qv")
                items0 = N_SUPPORT + g * G_Q
                gather_one(q_kT, 0, items0, G_Q, grp)
                gather_one(q_vT, 1, items0, G_Q, grp)
                qk3 = q_kT.rearrange("p m q a -> p m (q a)")
                qv3 = q_vT.rearrange("p m q a -> p m (q a)")
                qk8 = grp.tile([128, NDCH, C], fp8, tag="qk8", bufs=1)
                col_ln(qk3, C, qk8, grp, psum_pool, ptag="sumps")
                return qk8, qv3

            # ---------- Phase 2c: support V gather + transpose ----------
            # emitted inside group 0 (after its S-sums) so group-0 scores/sums
            # don't queue behind the 126 transposes on the PE
            def emit_support_v():
                for it0, n_it in WINS:
                    cols = n_it * T
                    nch = (cols + 127) // 128
                    wlo = (it0 * T) // 128
                    vw_t = sprep.tile([128, NDCH, nch * 128], bf16, tag="swin",
                                      name="vwin", bufs=1)
                    if cols < nch * 128:
                        nc.vector.memset(vw_t[:, :, cols:], 0.0)
                    gather_one(vw_t[:, :, :cols].rearrange("p m (n a) -> p m n a",
                                                           a=T),
                               1, it0, n_it, sprep)
                    for w in range(nch):
                        for dd in range(NDCH):
                            ps = pp_t.tile([128, 128], bf16, tag="tps")
                            nc.tensor.transpose(
                                ps, vw_t[:, dd, w * 128:(w + 1) * 128], ident)
                            if dd % 2 == 0:
                                nc.vector.tensor_copy(
                                    s_v[:, wlo + w, dd * 128:(dd + 1) * 128], ps)
                            else:
                                nc.scalar.activation(
                                    s_v[:, wlo + w, dd * 128:(dd + 1) * 128],
                                    ps, AF.Copy)

            # mask column index per (class, chunk)
            mcol = {}
            mi = 0
            for c in range(WAY):
                for (w, lo, hi) in spans[c]:
                    mcol[(c, w)] = mi
                    mi += 1

            # ---------- Phase 3: per-group query pipeline ----------
            TS, TB, TC, TA = 0, 1, 2, 3  # term slots in the packed tile

            def packed_sum(ps_tile, slot, rhs_chunks, first, last):
                """Accumulate masked column sums into ps_tile[32*slot] using a
                col-group tile_position; rhs_chunks = [(stat_col, mov)]."""
                out = ps_tile[32 * slot:32 * slot + 1]
                for i, (stat, rhs) in enumerate(rhs_chunks):
                    nc.tensor.matmul(out, stat, rhs, start=(first and i == 0),
                                     stop=(last and i == len(rhs_chunks) - 1),
                                     tile_position=(0, 32 * slot),
                                     skip_group_check=True)

            with tc.tile_pool(name="pp_sc", bufs=2, space="PSUM") as pp_sc, \
                 tc.tile_pool(name="pp_pr", bufs=2, space="PSUM") as pp_pr, \
                 tc.tile_pool(name="pp_row", bufs=2, space="PSUM") as pp_row:
                pend = q_gather(0, pp_row)
                for g in range(N_GROUPS):
                    qk8, qv3 = pend
                    if g + 1 < N_GROUPS:
                        pend = q_gather(g + 1, pp_row)

                    # scoresT + exp (fp8 DoubleRow over 9 d-chunks: 4 DR + 1)
                    exp_t = grp.tile([128, NWCH, C], bf16, tag="exp")
                    for w in range(NWCH):
                        ps = pp_sc.tile([128, C], f32, tag="scps")
                        for k2 in range(4):
                            nc.tensor.matmul(ps, sk8_chunk(w, slice(2 * k2, 2 * k2 + 2)),
                                             qk8[:, 2 * k2:2 * k2 + 2],
                                             start=(k2 == 0), stop=False,
                                             perf_mode=DR, skip_group_check=True)
                        nc.tensor.matmul(ps, sk8_chunk(w, 8), qk8[:, 8],
                                         start=False, stop=True,
                                         skip_group_check=True)
                        nc.scalar.activation(exp_t[:, w], ps, AF.Exp, scale=inv_sqrt)

                    packed = rows_pool.tile([WAY, 4, C], f32, tag="packed")

                    # A = ||q_v||^2 per column
                    qsq = grp.tile([128, NDCH, C], bf16, tag="lnsq", bufs=2)
                    nc.vector.tensor_mul(qsq, qv3, qv3)
                    ps_a = pp_row.tile([128, C], f32, tag="sumps", name="ps_a")
                    packed_sum(ps_a, 0, [(ones_sb, qsq[:, k]) for k in range(NDCH)],
                               True, True)
                    a_sb = rows_pool.tile([1, C], f32, tag="a_sb", bufs=1)
                    nc.vector.tensor_copy(a_sb, ps_a[0:1])
                    a5 = rows_pool.tile([WAY, C], f32, tag="a5", bufs=1)
                    nc.gpsimd.partition_broadcast(a5, a_sb)

                    def s_chunks(c):
                        return [(mask_sb[:, mcol[(c, w)]:mcol[(c, w)] + 1],
                                 exp_t[:, w]) for (w, lo, hi) in spans[c]]

                    def stage_rows(ps, rows):
                        """PSUM sum-tile -> SBUF (one lane-parallel copy), then
                        DMA rows {32j} into packed[class, term]."""
                        st = rows_pool.tile([128, C], f32, tag="stage", name="stage")
                        nc.scalar.activation(st, ps, AF.Copy)
                        st4 = st.rearrange("(j z) n -> j z n", z=32)
                        for j, (cc, term) in enumerate(rows):
                            nc.sync.dma_start(packed[cc:cc + 1, term],
                                              st4[j:j + 1, 0, :])

                    # S_0..S_3 packed in one PSUM tile (concurrent col-groups)
                    ps_s03 = pp_row.tile([128, C], f32, tag="sumps", name="ps_s03")
                    nchunks = max(len(spans[c]) for c in range(4))
                    for i in range(nchunks):
                        for c in range(4):
                            ch = s_chunks(c)
                            if i < len(ch):
                                packed_sum(ps_s03, c, [ch[i]], i == 0,
                                           i == len(ch) - 1)
                    stage_rows(ps_s03, [(0, TS), (1, TS), (2, TS), (3, TS)])

                    # remaining sum streams: S_4, then B_c/C_c per class,
                    # packed 4 per PSUM tile
                    ps_bc = pp_row.tile([128, C], f32, tag="sumps", name="ps_bc0")
                    packed_sum(ps_bc, 0, s_chunks(4), True, True)
                    pending = [(4, TS)]

                    if g == 0:
                        emit_support_v()

                    def bc_flush(force=False):
                        nonlocal ps_bc, pending
                        if pending and (force or len(pending) >= 3):
                            stage_rows(ps_bc, pending)
                            pending = []
                            if not force:
                                ps_bc = pp_row.tile([128, C], f32, tag="sumps",
                                                    name="ps_bc")
                    for c in range(WAY):
                        # prototypes: P[d, col] accumulated over class rows;
                        # boundary chunks use class-masked exp copies
                        movs = []
                        for (w, lo, hi) in spans[c]:
                            if lo == 0 and hi == 128:
                                movs.append((s_v[:, w], exp_t[:, w]))
                            else:
                                em = grp.tile([128, C], bf16, tag="emask",
                                              name="emask")
                                nc.vector.tensor_scalar(
                                    em, exp_t[:, w],
                                    mask_f[:, mcol[(c, w)]:mcol[(c, w)] + 1],
                                    0.0, ALU.mult, ALU.add)
                                movs.append((s_v[:, w], em))
                        pt = grp.tile([128, NDCH, C], bf16, tag="pt", bufs=1)
                        p2t = grp.tile([128, NDCH, C], bf16, tag="p2t", bufs=1)
                        for dd in range(NDCH):
                            ps_p = pp_pr.tile([128, C], f32, tag="prps")
                            for wi, (sv_w, mov) in enumerate(movs):
                                nc.tensor.matmul(ps_p,
                                                 sv_w[:, dd * 128:(dd + 1) * 128],
                                                 mov, start=(wi == 0),
                                                 stop=(wi == len(movs) - 1))
                            if dd % 2 == 0:
                                nc.vector.tensor_copy(p2t[:, dd], ps_p)
                            else:
                                nc.scalar.activation(p2t[:, dd], ps_p, AF.Copy)
                        nc.vector.tensor_mul(pt, p2t, qv3)
                        nc.vector.tensor_mul(p2t, p2t, p2t)
                        # interleave B_c / C_c chunk streams for PE concurrency
                        sb, sc = len(pending), len(pending) + 1
                        for k in range(NDCH):
                            packed_sum(ps_bc, sb, [(ones_sb, pt[:, k])],
                                       k == 0, k == NDCH - 1)
                            packed_sum(ps_bc, sc, [(ones_sb, p2t[:, k])],
                                       k == 0, k == NDCH - 1)
                        pending += [(c, TB), (c, TC)]
                        bc_flush()
                    bc_flush(force=True)

                    # dist = A - 2 B/S + C/S^2 ; logits = -sum_a dist / T
                    sinv = rows_pool.tile([WAY, C], f32, tag="sinv", bufs=1)
                    sscr = rows_pool.tile([WAY, C], f32, tag="sscr", bufs=1)
                    nc.vector.reciprocal_approx_accurate(sinv, packed[:, TS], sscr)
                    u = rows_pool.tile([WAY, C], f32, tag="u", bufs=1)
                    nc.vector.tensor_mul(u, packed[:, TC, :], sinv)
                    nc.vector.scalar_tensor_tensor(u, packed[:, TB, :], -2.0, u,
                                                   ALU.mult, ALU.add)
                    nc.vector.tensor_mul(u, u, sinv)
                    nc.vector.tensor_add(u, u, a5)
                    u4 = u.rearrange("w (q a) -> w q a", a=T)
                    red = rows_pool.tile([WAY, G_Q], f32, tag="red", bufs=1)
                    nc.vector.reduce_sum(red, u4, mybir.AxisListType.X)
                    nc.scalar.activation(logits5[:, g * G_Q:(g + 1) * G_Q], red,
                                         AF.Copy, scale=-1.0 / T)

            rows_cm.__exit__(None, None, None)
            grp_cm.__exit__(None, None, None)
            nc.sync.dma_start(out_d.rearrange("q c -> c q"), logits5)

    nc.compile()
    return nc


def kernel(support_set, support_labels, queries, k_w, k_b, v_w, v_b, ln_g, ln_b):
    import concourse.bass_utils as bass_utils

    support_set = np.asarray(support_set, dtype=np.float32)
    queries = np.asarray(queries, dtype=np.float32)
    labels = np.asarray(support_labels, dtype=np.int32)
    k_w = np.asarray(k_w, dtype=np.float32)
    v_w = np.asarray(v_w, dtype=np.float32)
    k_b = np.asarray(k_b, dtype=np.float32)
    v_b = np.asarray(v_b, dtype=np.float32)
    ln_g = np.asarray(ln_g, dtype=np.float32)
    ln_b = np.asarray(ln_b, dtype=np.float32)

    pe = _pos_encoding()
    s = support_set + pe[None]
    q = queries + pe[None]
    order = np.argsort(labels, kind="stable")
    counts = np.bincount(labels, minlength=WAY)
    s_sorted = s[order]
    trivial_gb = bool(np.all(ln_g == 1.0) and np.all(ln_b == 0.0))
    has_bias = bool(np.any(k_b != 0.0) or np.any(v_b != 0.0))

    key = (tuple(int(x) for x in counts), trivial_gb, has_bias)
    if key not in _CACHE:
        _CACHE[key] = _build_kernel(counts, trivial_gb, has_bias)
    nc = _CACHE[key]

    W = np.zeros((IN_DIM, 6 * OUT_DIM), np.float32)
    for j in range(TSS):
        W[:, j * OUT_DIM:(j + 1) * OUT_DIM] = k_w[j * IN_DIM:(j + 1) * IN_DIM]
        W[:, (TSS + j) * OUT_DIM:(TSS + j + 1) * OUT_DIM] = v_w[j * IN_DIM:(j + 1) * IN_DIM]
    w_perm = np.ascontiguousarray(
        W.reshape(NKCH, 128, NMB, 128).transpose(1, 2, 0, 3)).astype(FP8)

    offs, spans = _geom(counts)
    nmask = sum(len(sp) for sp in spans)
    masks = np.zeros((128, nmask), np.float32)
    mi = 0
    for c in range(WAY):
        for (w, lo, hi) in spans[c]:
            masks[lo:hi, mi] = 1.0
            mi += 1
    masks = masks.astype(BF16)

    base = {"w": w_perm, "masks": masks}
    if has_bias:
        pb = np.zeros((128, NMB), np.float32)
        for m in range(NMB):
            blk, dd = divmod(m, NDCH)
            bias = k_b if blk < TSS else v_b
            pb[:, m] = bias[dd * 128:(dd + 1) * 128] / TSS
        base["pb"] = pb
    if not trivial_gb:
        base["lng"] = np.ascontiguousarray(ln_g.reshape(NDCH, 128).T).astype(BF16)
        base["lnb"] = np.ascontiguousarray(ln_b.reshape(NDCH, 128).T).astype(BF16)

    in_maps = []
    for core in range(N_CORES):
        qs = q[core * NQL:(core + 1) * NQL]
        X = np.concatenate([s_sorted.reshape(-1, IN_DIM), qs.reshape(-1, IN_DIM)], 0)
        x_perm = np.ascontiguousarray(
            X.T.reshape(NKCH, 128, NX).transpose(1, 0, 2)).astype(FP8)
        in_maps.append(dict(base, x=x_perm))

    global _LAST_IN_MAPS
    _LAST_IN_MAPS = in_maps
    res = bass_utils.run_bass_kernel_spmd(nc, in_maps, core_ids=list(range(N_CORES)))
    return np.concatenate([res.results[i]["out"] for i in range(N_CORES)], 0)


_LAST_IN_MAPS = None
